# revision 24
# baseline (speedup 1.0000x reference)
"""GCN layer (GCNConv + relu + dense + relu) on 8 Trainium2 NeuronCores.

Strategy (v3 — supply-limited gather, 64B elems, decoupled consumers)
---------------------------------------------------------------------
Math: out = relu(relu(GCNConv(x)) @ W_dense + b_dense) with
GCNConv(x)[v] = dinv[v] * sum_{e: src->v} dinv[src] * (x W_gcn)[src] + b_gcn
(self-loops included as ordinary edges; dinv = rsqrt(indegree incl. self).)

Measured on HW (microbench): the SWDGE dma_gather path sustains ~2.4ns/idx
with 4 queues regardless of elem size (descriptor-supply limited on the
Pool engine), so launch B's floor is idx-count * 2.4ns.  v3 therefore:
  * packs slots tighter (64-slot cell alignment instead of 64-unit
    instruction padding): ~425k slots/core vs 500k in v2;
  * gathers 64B elems (elem_size=32 bf16, elem_step=128 -> 256B stride;
    the %256 elem-size rule is an over-conservative bass assert — a local
    emitter mirrors bass.BassGpSimd.dma_gather without it; verified
    bit-exact on HW) — 4x less DMA-engine time and 4x smaller msg tiles,
    buying pool depth so gathers never wait on consumers;
  * slot stream is chunk-major with per-(tile,chunk) cells: each cell's
    segment-sum closes as a short PSUM group (matmul pieces at 64-part
    granularity), then a DVE add folds it into a persistent fp32 SBUF
    accumulator [32, 12544] — no long-lived PSUM banks, so gathers,
    one-hots (DVE), and matmuls (PE) all pipeline freely under the Pool
    roof.
Launch A (unchanged math): per core, g = dinv_row * (x @ W_gcn) as bf16
rows; host pads them into the [100000, 128]-strided table (only cols 0:32
are read by the 64B gathers).  dinv ships node-linear via DRAM.
"""

import sys

if "/opt/trn_rl_repo" not in sys.path:
    sys.path.insert(0, "/opt/trn_rl_repo")

from dataclasses import dataclass

import numpy as np

import concourse.bacc as bacc
import concourse.bass as bass
import concourse.mybir as mybir
from concourse import tile
from concourse.bass_utils import run_bass_kernel_spmd


@dataclass(frozen=True)
class Cfg:
    n_cores: int = 8
    nloc: int = 12500
    ntiles: int = 98              # 128-row dst tiles per core (12544 padded)
    in_dim: int = 128
    net_dim: int = 32
    row: int = 128                # table row stride (bf16) = 256B
    gel: int = 32                 # gathered elem (bf16) = 64B
    n_chunks: int = 4             # src chunks of 25000 (int16 gather idx)
    chunk: int = 25000
    ni: int = 1024                # idxs per gather instruction (hard cap)

    @property
    def npad(self):
        return self.ntiles * 128  # 12544

    @property
    def n(self):
        return self.nloc * self.n_cores


FULL = Cfg()
assert FULL.n == 100000 and FULL.chunk * FULL.n_chunks == FULL.n


def _f32(x):
    return np.ascontiguousarray(x, dtype=np.float32)


def wrap16(a):
    """Index array [n] -> [128, n//16] int16 layout dma_gather expects."""
    assert a.size % 16 == 0
    w = a.reshape(-1, 16).T
    return np.ascontiguousarray(np.tile(w, (8, 1)), dtype=np.int16)


def gather64(nc, out_ap, in_ap, idxs_ap, num_idxs, elem_size, elem_step, queue_num):
    """nc.gpsimd.dma_gather minus the `elem_size_bytes % 256 == 0` assert.

    Mirrors bass.BassGpSimd.dma_gather for the DRAM-source, non-transpose,
    gen_mode=0 case.  elem_step (the idx stride) must still be a multiple
    of 256B; elem_size may be any positive size (64B verified on HW).
    """
    gp = nc.gpsimd
    gp._assert_queue_num(queue_num)
    assert idxs_ap.dtype == mybir.dt.int16
    assert in_ap.dtype == out_ap.dtype
    from concourse import ap_utils
    from concourse.bass import exact_div

    assert ap_utils.ap_is_contiguous(in_ap.ap[1:])
    assert ap_utils.ap_is_contiguous(out_ap.ap[1:])
    assert ap_utils.ap_is_contiguous(idxs_ap.ap[1:])
    assert in_ap.ap[-1][1] == out_ap.ap[-1][1] == elem_size
    assert out_ap.ap[0][1] * out_ap.ap[1][1] == ((num_idxs + 127) // 128) * 128
    assert in_ap.ap[0][0] == elem_step
    stride_bytes = elem_step * mybir.dt.size(in_ap.dtype)
    stride_bytes_256 = exact_div(stride_bytes, 256)
    assert stride_bytes_256 < 256
    _in_ap = gp.lower_ap_dma(in_ap, for_custom_bir_dma=True)
    _idxs_ap = gp.lower_ap(idxs_ap)
    _out_ap = gp.lower_ap(out_ap)
    return gp.add_instruction(
        mybir.InstDMAGatherAnt(
            name=nc.get_next_instruction_name(),
            ins=[*_in_ap, _idxs_ap, gp.lower_val_access(gp.to_reg(num_idxs))],
            outs=[_out_ap],
            transpose=False,
            num_idxs=num_idxs,
            elem_size=elem_size,
            stride_bytes_256=stride_bytes_256,
            gen_mode=0,
            single_packet=True,
            queue_num=queue_num,
            sbuf_tokens_per_rank=0,
            sbuf_free_dim_per_rank=0,
            sbuf_free_dim_pad_per_rank=0,
            sbuf_byte_offset=0,
        )
    )


def _emit_dinv(nc, pool, deg_d, p, w, name):
    """deg (int32 [p, w]) -> dinv = 1/sqrt(deg) with a Newton step."""
    deg_t = pool.tile([p, w], mybir.dt.int32, name=f"{name}_i", tag="dinv_i")
    degf_t = pool.tile([p, w], mybir.dt.float32, name=f"{name}_f", tag="dinv_f")
    r_t = pool.tile([p, w], mybir.dt.float32, name=f"{name}_r", tag="dinv_r")
    s_t = pool.tile([p, w], mybir.dt.float32, name=f"{name}_s", tag="dinv_s")
    dinv_t = pool.tile([p, w], mybir.dt.float32, name=f"{name}_v", tag="dinv_v")
    nc.sync.dma_start(out=deg_t[:], in_=deg_d[:])
    nc.vector.tensor_copy(out=degf_t[:], in_=deg_t[:])
    nc.vector.reciprocal(out=r_t[:], in_=degf_t[:])
    # Newton: r <- r * (2 - d * r) computed as -(r * (d*r - 2))
    nc.vector.tensor_tensor(out=s_t[:], in0=degf_t[:], in1=r_t[:], op=mybir.AluOpType.mult)
    nc.vector.tensor_scalar_add(out=s_t[:], in0=s_t[:], scalar1=-2.0)
    nc.vector.tensor_tensor(out=s_t[:], in0=s_t[:], in1=r_t[:], op=mybir.AluOpType.mult)
    nc.vector.tensor_scalar_mul(out=s_t[:], in0=s_t[:], scalar1=-1.0)
    nc.scalar.sqrt(dinv_t[:], s_t[:])
    return dinv_t


# ---------------------------------------------------------------- layout


class Layout:
    """Slot/instruction structure for launch B (static across cores).

    Slot stream per chunk k: cells (t, k) for t = 0..97, each padded to a
    64-slot boundary; the stream is cut into 1024-idx instructions at
    arbitrary cell positions.  Per (t, k) the cell maps to a static list
    of (instr, batch-in-instr, lo, hi) 64-granular matmul pieces.
    """

    def __init__(self, cfg: Cfg, ncell):
        self.cfg = cfg
        self.ncell = ncell            # [ntiles, n_chunks] slots per cell (64-mult)
        T, K, NI = cfg.ntiles, cfg.n_chunks, cfg.ni
        self.cell_start = np.zeros((T, K), dtype=np.int64)  # global slot idx
        self.k_slots = []             # padded stream length per chunk
        self.k_base = []              # global slot base per chunk
        self.instrs = []              # (k, slot0_in_k, nidx, batch_base)
        self.tk_pieces = {}
        base = 0
        B = 0
        for k in range(K):
            s = 0
            for t in range(T):
                self.cell_start[t, k] = base + s
                s += int(ncell[t, k])
            spad = ((s + NI - 1) // NI) * NI
            self.k_base.append(base)
            self.k_slots.append(spad)
            for i in range(spad // NI):
                self.instrs.append((k, i * NI, NI, B))
                B += NI // 128
            base += spad
        self.NS = base
        self.NB = B
        self.NI_n = len(self.instrs)
        ipk = [self.k_slots[k] // NI for k in range(K)]
        i_base = np.cumsum([0] + ipk)
        for k in range(K):
            for t in range(T):
                s0 = int(self.cell_start[t, k]) - self.k_base[k]
                s1 = s0 + int(ncell[t, k])
                pieces = []
                s = s0
                while s < s1:
                    b = s // 128
                    lo = s - b * 128
                    hi = min(s1 - b * 128, 128)
                    i = int(i_base[k]) + b * 128 // NI
                    bi = b - (b * 128 // NI) * (NI // 128)
                    assert lo in (0, 64) and hi in (64, 128)
                    pieces.append((i, bi, lo, hi))
                    s = b * 128 + hi
                self.tk_pieces[(t, k)] = pieces


def make_layout(cfg: Cfg, counts):
    """counts: [n_cores, ntiles, n_chunks] edge counts -> static Layout."""
    mx = counts.max(axis=0)
    ncell = np.maximum((mx + 63) // 64, 1) * 64
    return Layout(cfg, ncell)


# ---------------------------------------------------------------- launch A


def build_launch_a(cfg: Cfg):
    nc = bacc.Bacc(
        "TRN2", target_bir_lowering=False, debug=False, num_devices=cfg.n_cores
    )
    T, K, F = cfg.ntiles, cfg.in_dim, cfg.net_dim
    xT_d = nc.dram_tensor("xT", [T, K, 128], mybir.dt.bfloat16, kind="ExternalInput")
    w_d = nc.dram_tensor("w", [K, F], mybir.dt.bfloat16, kind="ExternalInput")
    deg_d = nc.dram_tensor("deg", [128, T], mybir.dt.int32, kind="ExternalInput")
    g_d = nc.dram_tensor("g", [cfg.npad, F], mybir.dt.bfloat16, kind="ExternalOutput")
    dv_d = nc.dram_tensor("dinv1", [cfg.npad], mybir.dt.float32, kind="ExternalOutput")

    with tile.TileContext(nc) as tc:
        with (
            tc.tile_pool(name="const", bufs=1) as cpool,
            tc.tile_pool(name="xin", bufs=4) as xpool,
            tc.tile_pool(name="gout", bufs=4) as gpool,
            tc.tile_pool(name="ph", bufs=4, space="PSUM") as php,
        ):
            w_t = cpool.tile([K, F], mybir.dt.bfloat16)
            nc.sync.dma_start(out=w_t[:], in_=w_d[:])
            dinv_t = _emit_dinv(nc, cpool, deg_d, 128, T, "dinv")

            GA = 8
            for q in range((T + GA - 1) // GA):
                t0 = q * GA
                tn = min(GA, T - t0)
                xT_t = xpool.tile([128, GA, 128], mybir.dt.bfloat16, tag="x")
                nc.sync.dma_start(
                    out=xT_t[:, :tn, :],
                    in_=xT_d[t0 : t0 + tn].rearrange("a f c -> f a c"),
                )
                g_t = gpool.tile([128, GA, F], mybir.dt.bfloat16, tag="g")
                for j in range(tn):
                    h_p = php.tile([128, F], mybir.dt.float32, tag="h")
                    nc.tensor.matmul(
                        h_p[:], xT_t[:, j, :], w_t[:], start=True, stop=True
                    )
                    # per-partition dinv scale on DVE (ACT copies are ~7x
                    # slower and were launch A's wall)
                    nc.vector.tensor_tensor(
                        out=g_t[:, j, :],
                        in0=h_p[:],
                        in1=dinv_t[:, t0 + j : t0 + j + 1].broadcast_to((128, F)),
                        op=mybir.AluOpType.mult,
                    )
                nc.sync.dma_start(
                    out=g_d[t0 * 128 : (t0 + tn) * 128, :].rearrange(
                        "(a p) c -> p a c", p=128
                    ),
                    in_=g_t[:, :tn, :],
                )

            nc.sync.dma_start(
                out=dv_d.ap().rearrange("(t p) -> p t", p=128), in_=dinv_t[:]
            )
    nc.compile()
    return nc


# ---------------------------------------------------------------- launch B


def build_launch_b(cfg: Cfg, lay: Layout):
    nc = bacc.Bacc(
        "TRN2",
        target_bir_lowering=False,
        debug=False,
        num_devices=cfg.n_cores,
        num_swdge_queues=4,
    )
    F, GEL, NI = cfg.net_dim, cfg.gel, cfg.ni
    NB, NIN = lay.NB, lay.NI_n
    NBPI = NI // 128  # batches per instr

    g_d = nc.dram_tensor("g", [cfg.n, cfg.row], mybir.dt.bfloat16, kind="ExternalInput")
    src_d = nc.dram_tensor(
        "src_i", [NIN, 128, NI // 16], mybir.dt.int16, kind="ExternalInput"
    )
    dstv_d = nc.dram_tensor("dstv", [128, NB], mybir.dt.bfloat16, kind="ExternalInput")
    iota_d = nc.dram_tensor("iota", [128, 128], mybir.dt.bfloat16, kind="ExternalInput")
    dv1_d = nc.dram_tensor("dinv1", [cfg.npad], mybir.dt.float32, kind="ExternalInput")
    bg_d = nc.dram_tensor("bg", [F, 1], mybir.dt.float32, kind="ExternalInput")
    wd_d = nc.dram_tensor("wd", [F, F], mybir.dt.float32, kind="ExternalInput")
    bd_d = nc.dram_tensor("bd", [F, 1], mybir.dt.float32, kind="ExternalInput")
    out_d = nc.dram_tensor("out", [F, cfg.npad], mybir.dt.float32, kind="ExternalOutput")

    # instr id -> list of (t, pieces) fully contained; a cell's pieces may
    # span two instrs only via its batch fragments, which are per-instr
    # anyway, so group pieces by their instr.
    by_instr = [[] for _ in range(NIN)]
    for k in range(cfg.n_chunks):
        for t in range(cfg.ntiles):
            for (i, bi, lo, hi) in lay.tk_pieces[(t, k)]:
                by_instr[i].append((t, k, bi, lo, hi))

    with tile.TileContext(nc) as tc:
        with (
            tc.tile_pool(name="const", bufs=1) as cpool,
            tc.tile_pool(name="idx", bufs=24) as ipool,
            tc.tile_pool(name="msg", bufs=40) as mpool,
            tc.tile_pool(name="oh", bufs=16) as opool,
            tc.tile_pool(name="epi", bufs=2) as epool,
            tc.tile_pool(name="acc", bufs=6, space="PSUM") as apool,
            tc.tile_pool(name="h2", bufs=2, space="PSUM") as hpool,
        ):
            iota_t = cpool.tile([128, 128], mybir.dt.bfloat16)
            nc.sync.dma_start(out=iota_t[:], in_=iota_d[:])
            dstv_t = cpool.tile([128, NB], mybir.dt.bfloat16)
            nc.sync.dma_start(out=dstv_t[:], in_=dstv_d[:])
            bg_t = cpool.tile([F, 1], mybir.dt.float32)
            nc.sync.dma_start(out=bg_t[:], in_=bg_d[:])
            wd_t = cpool.tile([F, F], mybir.dt.float32)
            nc.sync.dma_start(out=wd_t[:], in_=wd_d[:])
            bd_t = cpool.tile([F, 1], mybir.dt.float32)
            nc.sync.dma_start(out=bd_t[:], in_=bd_d[:])
            acc_t = cpool.tile([F, cfg.npad], mybir.dt.float32, name="accsb")
            nc.vector.memset(acc_t[:], 0.0)
            # dinv feature-major [32, npad], loaded after the first gathers
            # are queued so the preamble doesn't delay them.
            dinv_t = cpool.tile([F, cfg.npad], mybir.dt.float32, name="dinvfm")

            # per-(t,k) accumulation state: psum tile + piece countdown
            npieces = {
                (t, k): len(lay.tk_pieces[(t, k)])
                for k in range(cfg.n_chunks)
                for t in range(cfg.ntiles)
            }
            done = {key: 0 for key in npieces}
            psum_of = {}

            # epilogue blocks (4 tiles each), emitted as soon as all their
            # tiles' cells have folded into acc so the tail doesn't serialize
            # after the last gather
            BL = 512
            NQ = (cfg.npad + BL - 1) // BL
            tile_cells = [0] * cfg.ntiles
            blk_left = [min(4 * (q + 1), cfg.ntiles) - 4 * q for q in range(NQ)]
            emitted = [False] * NQ

            def epilogue_block(q):
                c0 = q * BL
                W = min(BL, cfg.npad - c0)
                h1_t = epool.tile([F, BL], mybir.dt.float32, tag="h1", name="h1")
                nc.vector.tensor_tensor(
                    out=h1_t[:, :W],
                    in0=acc_t[:, c0 : c0 + W],
                    in1=dinv_t[:, c0 : c0 + W],
                    op=mybir.AluOpType.mult,
                )
                r1_t = epool.tile([F, BL], mybir.dt.float32, tag="r1", name="r1")
                nc.scalar.activation(
                    r1_t[:, :W], h1_t[:, :W],
                    mybir.ActivationFunctionType.Relu, bias=bg_t[:],
                )
                h2_p = hpool.tile([F, BL], mybir.dt.float32, tag="h2", name="h2")
                nc.tensor.matmul(
                    h2_p[:, :W], wd_t[:], r1_t[:, :W], start=True, stop=True
                )
                o_t = epool.tile([F, BL], mybir.dt.float32, tag="o", name="o")
                nc.scalar.activation(
                    o_t[:, :W], h2_p[:, :W],
                    mybir.ActivationFunctionType.Relu, bias=bd_t[:],
                )
                nc.sync.dma_start(out=out_d[:, c0 : c0 + W], in_=o_t[:, :W])
                emitted[q] = True

            for i, (k, s0, nidx, bb) in enumerate(lay.instrs):
                idx_t = ipool.tile([128, NI // 16], mybir.dt.int16, tag="idx")
                nc.sync.dma_start(out=idx_t[:], in_=src_d[i])
                msg_t = mpool.tile([128, NBPI, GEL], mybir.dt.bfloat16, tag="m")
                gather64(
                    nc,
                    msg_t[:],
                    g_d[k * cfg.chunk : (k + 1) * cfg.chunk, 0:GEL],
                    idx_t[:],
                    nidx,
                    GEL,
                    cfg.row,
                    queue_num=i % 4,
                )
                oh_t = opool.tile([128, NBPI, 128], mybir.dt.bfloat16, tag="oh")
                nc.vector.tensor_tensor(
                    out=oh_t[:],
                    in0=iota_t[:].unsqueeze(1).broadcast_to((128, NBPI, 128)),
                    in1=dstv_t[:, bb : bb + NBPI]
                    .unsqueeze(2)
                    .broadcast_to((128, NBPI, 128)),
                    op=mybir.AluOpType.is_equal,
                )

                if i == 1:
                    for f in range(F):
                        nc.sync.dma_start(
                            out=dinv_t[f : f + 1, :], in_=dv1_d.ap().unsqueeze(0)
                        )

                for (t, k2, bi, lo, hi) in by_instr[i]:
                    key = (t, k2)
                    if key not in psum_of:
                        # full PSUM bank per open accumulator: a start=True
                        # while another group is open in the same bank
                        # destroys it on HW.
                        psum_of[key] = apool.tile(
                            [128, 512], mybir.dt.float32, tag="acc",
                            name=f"acc_{t}_{k2}",
                        )
                    p_t = psum_of[key]
                    first = done[key] == 0
                    done[key] += 1
                    last = done[key] == npieces[key]
                    nc.tensor.matmul(
                        p_t[0:F, 0:128],
                        msg_t[lo:hi, bi, :],
                        oh_t[lo:hi, bi, :],
                        start=first,
                        stop=last,
                    )
                    if last:
                        nc.vector.tensor_tensor(
                            out=acc_t[:, t * 128 : (t + 1) * 128],
                            in0=acc_t[:, t * 128 : (t + 1) * 128],
                            in1=p_t[0:F, 0:128],
                            op=mybir.AluOpType.add,
                        )
                        del psum_of[key]
                        tile_cells[t] += 1
                        if tile_cells[t] == cfg.n_chunks:
                            q = t // 4
                            blk_left[q] -= 1
                            if blk_left[q] == 0:
                                epilogue_block(q)

            for q in range(NQ):
                if not emitted[q]:
                    epilogue_block(q)
    nc.compile()
    return nc


# ---------------------------------------------------------------- host side


def host_prep(x, edge_index, W_gcn, b_gcn, W_dense, b_dense, cfg: Cfg):
    n, nloc = cfg.n, cfg.nloc
    row = np.asarray(edge_index[0]).astype(np.int64)
    col = np.asarray(edge_index[1]).astype(np.int64)
    deg = (np.bincount(col, minlength=n) + 1).astype(np.int32)  # + self-loop

    W_gcn = _f32(W_gcn)
    b_gcn = _f32(b_gcn).reshape(cfg.net_dim, 1)
    W_dense = _f32(W_dense)
    b_dense = _f32(b_dense).reshape(cfg.net_dim, 1)
    x = _f32(x)

    import ml_dtypes

    iota = np.tile(np.arange(128), (128, 1)).astype(ml_dtypes.bfloat16)

    # ---- per-core edge sets (dst-sharded) + self loops.  Node -> tile
    # assignment is rebalanced per core (greedy 4-D bin packing on per-chunk
    # degrees) so the (tile, chunk) cells are near-uniform: the layout pads
    # each cell to the 64-aligned max across cores, so balance = fewer pad
    # slots = fewer gather descriptors (the kernel's wall).
    owner = col // nloc
    per_core = []
    perms = []
    counts = np.zeros((cfg.n_cores, cfg.ntiles, cfg.n_chunks), dtype=np.int64)
    for c in range(cfg.n_cores):
        m = owner == c
        srcs = row[m]
        dstl = col[m] - c * nloc
        loop = np.arange(nloc, dtype=np.int64)
        srcs = np.concatenate([srcs, loop + c * nloc])
        dstl = np.concatenate([dstl, loop])
        k = srcs // cfg.chunk

        d4 = np.zeros((nloc, cfg.n_chunks), dtype=np.int64)
        np.add.at(d4, (dstl, k), 1)
        # LPT on total degree (balances tile totals), then chunk-mix swaps
        order_n = np.argsort(-d4.sum(axis=1), kind="stable")
        cell = np.zeros((cfg.ntiles, cfg.n_chunks), dtype=np.int64)
        fill = np.zeros(cfg.ntiles, dtype=np.int64)
        tile_of = np.empty(nloc, dtype=np.int64)
        BIG = 1 << 30
        for i in order_n:
            cand = cell.sum(axis=1) + np.where(fill >= 128, BIG, 0)
            t_best = int(np.argmin(cand))
            tile_of[i] = t_best
            fill[t_best] += 1
            cell[t_best] += d4[i]

        # swap-descent: push the worst (tile, chunk) cells toward the mean
        # (each pad-to-64 slot above the cross-core max is a wasted gather
        # descriptor on the kernel's critical path)
        nodes_by_tile = [list(np.nonzero(tile_of == t)[0]) for t in range(cfg.ntiles)]
        rng = np.random.default_rng(c)
        # push every cell under the next 64-slot boundary
        target = (int(np.ceil(cell.sum() / cell.size)) // 64 + 1) * 64 - 8
        stuck = set()
        for _ in range(15000):
            flat = cell.copy()
            for (ts, ks) in stuck:
                flat[ts, ks] = -1
            t1, k1 = np.unravel_index(int(np.argmax(flat)), cell.shape)
            if cell[t1, k1] <= target:
                break
            na = nodes_by_tile[t1]
            sa = np.argsort(-(2 * d4[na, k1] - d4[na].sum(axis=1)))
            a_cands = np.asarray(na)[sa[:8]]
            improved = False
            for t2 in np.argsort(cell[:, k1])[:16]:
                if improved:
                    break
                nb = nodes_by_tile[t2]
                sb = np.argsort(2 * d4[nb, k1] - d4[nb].sum(axis=1))
                b_cands = np.asarray(nb)[sb[:8]]
                for a in a_cands:
                    if improved:
                        break
                    for b in b_cands:
                        c1n = cell[t1] - d4[a] + d4[b]
                        c2n = cell[t2] - d4[b] + d4[a]
                        if max(c1n.max(), c2n.max()) < cell[t1, k1]:
                            cell[t1] = c1n
                            cell[t2] = c2n
                            nodes_by_tile[t1].remove(a)
                            nodes_by_tile[t2].remove(b)
                            nodes_by_tile[t1].append(b)
                            nodes_by_tile[t2].append(a)
                            tile_of[a] = t2
                            tile_of[b] = t1
                            improved = True
                            break
            if not improved:
                stuck.add((t1, k1))
                if len(stuck) > 400:
                    break

        newpos = np.empty(nloc, dtype=np.int64)
        fill[:] = 0
        for i in range(nloc):
            t = tile_of[i]
            newpos[i] = t * 128 + fill[t]
            fill[t] += 1
        perms.append(newpos)

        dstl = newpos[dstl]
        t = dstl >> 7
        np.add.at(counts[c], (t, k), 1)
        per_core.append((srcs, dstl, t, k))

    lay = make_layout(cfg, counts)
    lay.perms = perms

    in_a, in_b = [], []
    for c in range(cfg.n_cores):
        srcs, dstl, t, k = per_core[c]
        newpos = perms[c]
        dpad = np.ones(cfg.npad, dtype=np.int32)
        dpad[newpos] = deg[c * nloc : (c + 1) * nloc]
        deg_a = np.ascontiguousarray(dpad.reshape(cfg.ntiles, 128).T)  # [128, T]

        xpad = np.zeros((cfg.npad, cfg.in_dim), dtype=np.float32)
        xpad[newpos] = x[c * nloc : (c + 1) * nloc]
        xT3 = np.ascontiguousarray(
            xpad.reshape(cfg.ntiles, 128, cfg.in_dim).transpose(0, 2, 1)
        ).astype(ml_dtypes.bfloat16)
        in_a.append({"xT": xT3, "w": W_gcn.astype(ml_dtypes.bfloat16), "deg": deg_a})

        # ---- slot assignment into the v3 stream
        src_slots = np.zeros(lay.NS, dtype=np.int64)
        dst_slots = np.full(lay.NS, -1.0, dtype=np.float64)
        order = np.lexsort((t, k))
        ts, ks = t[order], k[order]
        so = (srcs[order] - ks * cfg.chunk).astype(np.int64)
        do = (dstl[order] & 127).astype(np.int64)
        grp = ks * cfg.ntiles + ts
        uq, starts_, cnts_ = np.unique(grp, return_index=True, return_counts=True)
        rank = np.arange(grp.size) - np.repeat(starts_, cnts_)
        slot = lay.cell_start[ts, ks] + rank
        src_slots[slot] = so
        dst_slots[slot] = do

        src_i = np.zeros((lay.NI_n, 128, cfg.ni // 16), dtype=np.int16)
        dstv = np.empty((128, lay.NB), dtype=np.float64)
        for i, (kk, s0, nidx, bb) in enumerate(lay.instrs):
            g0 = lay.k_base[kk] + s0
            seg = src_slots[g0 : g0 + nidx]
            src_i[i] = wrap16(seg)
            dseg = dst_slots[g0 : g0 + nidx]
            nbt = nidx // 128
            dstv[:, bb : bb + nbt] = dseg.reshape(nbt, 128).T
        dstv = np.ascontiguousarray(dstv).astype(ml_dtypes.bfloat16)

        in_b.append(
            {
                "src_i": src_i,
                "dstv": dstv,
                "iota": iota,
                "bg": b_gcn,
                "wd": W_dense,
                "bd": b_dense,
            }
        )
    return in_a, in_b, lay


def assemble_table(res_a, cfg: Cfg, lay: Layout):
    import ml_dtypes

    tab = np.zeros((cfg.n, cfg.row), dtype=ml_dtypes.bfloat16)
    for c in range(cfg.n_cores):
        tab[c * cfg.nloc : (c + 1) * cfg.nloc, 0 : cfg.net_dim] = res_a[c]["g"][
            lay.perms[c]
        ]
    return tab


def assemble_out(res_b, cfg: Cfg, lay: Layout):
    return np.ascontiguousarray(
        np.concatenate(
            [res_b[c]["out"].T[lay.perms[c]] for c in range(cfg.n_cores)], axis=0
        )
    ).astype(np.float32)


def _add_table(in_b, table, res_a, cfg: Cfg):
    for c, m in enumerate(in_b):
        m["g"] = table
        m["dinv1"] = np.ascontiguousarray(res_a[c]["dinv1"], dtype=np.float32)


def kernel(x, edge_index, W_gcn, b_gcn, W_dense, b_dense):
    cfg = FULL
    in_a, in_b, lay = host_prep(x, edge_index, W_gcn, b_gcn, W_dense, b_dense, cfg)
    nc_a = build_launch_a(cfg)
    nc_b = build_launch_b(cfg, lay)
    core_ids = list(range(cfg.n_cores))
    res_a = run_bass_kernel_spmd(nc_a, in_a, core_ids).results
    table = assemble_table(res_a, cfg, lay)
    _add_table(in_b, table, res_a, cfg)
    res_b = run_bass_kernel_spmd(nc_b, in_b, core_ids).results
    return assemble_out(res_b, cfg, lay)


# revision 25
# speedup vs baseline: 1.1652x; 1.1652x over previous
"""GCN layer (GCNConv + relu + dense + relu) on 8 Trainium2 NeuronCores.

Strategy (v3 — supply-limited gather, 64B elems, decoupled consumers)
---------------------------------------------------------------------
Math: out = relu(relu(GCNConv(x)) @ W_dense + b_dense) with
GCNConv(x)[v] = dinv[v] * sum_{e: src->v} dinv[src] * (x W_gcn)[src] + b_gcn
(self-loops included as ordinary edges; dinv = rsqrt(indegree incl. self).)

Measured on HW (microbench): the SWDGE dma_gather path sustains ~2.4ns/idx
with 4 queues regardless of elem size (descriptor-supply limited on the
Pool engine), so launch B's floor is idx-count * 2.4ns.  v3 therefore:
  * packs slots tighter (64-slot cell alignment instead of 64-unit
    instruction padding): ~425k slots/core vs 500k in v2;
  * gathers 64B elems (elem_size=32 bf16, elem_step=128 -> 256B stride;
    the %256 elem-size rule is an over-conservative bass assert — a local
    emitter mirrors bass.BassGpSimd.dma_gather without it; verified
    bit-exact on HW) — 4x less DMA-engine time and 4x smaller msg tiles,
    buying pool depth so gathers never wait on consumers;
  * slot stream is chunk-major with per-(tile,chunk) cells: each cell's
    segment-sum closes as a short PSUM group (matmul pieces at 64-part
    granularity), then a DVE add folds it into a persistent fp32 SBUF
    accumulator [32, 12544] — no long-lived PSUM banks, so gathers,
    one-hots (DVE), and matmuls (PE) all pipeline freely under the Pool
    roof.
Launch A (unchanged math): per core, g = dinv_row * (x @ W_gcn) as bf16
rows; host pads them into the [100000, 128]-strided table (only cols 0:32
are read by the 64B gathers).  dinv ships node-linear via DRAM.
"""

import sys

if "/opt/trn_rl_repo" not in sys.path:
    sys.path.insert(0, "/opt/trn_rl_repo")

from dataclasses import dataclass

import numpy as np

import concourse.bacc as bacc
import concourse.bass as bass
import concourse.mybir as mybir
from concourse import tile
from concourse.bass_utils import run_bass_kernel_spmd


@dataclass(frozen=True)
class Cfg:
    n_cores: int = 8
    nloc: int = 12500
    ntiles: int = 98              # 128-row dst tiles per core (12544 padded)
    in_dim: int = 128
    net_dim: int = 32
    row: int = 128                # table row stride (bf16) = 256B
    gel: int = 32                 # gathered elem (bf16) = 64B
    n_chunks: int = 4             # src chunks of 25000 (int16 gather idx)
    chunk: int = 25000
    ni: int = 1024                # idxs per gather instruction (hard cap)

    @property
    def npad(self):
        return self.ntiles * 128  # 12544

    @property
    def n(self):
        return self.nloc * self.n_cores


FULL = Cfg()
assert FULL.n == 100000 and FULL.chunk * FULL.n_chunks == FULL.n


def _f32(x):
    return np.ascontiguousarray(x, dtype=np.float32)


def wrap16(a):
    """Index array [n] -> [128, n//16] int16 layout dma_gather expects."""
    assert a.size % 16 == 0
    w = a.reshape(-1, 16).T
    return np.ascontiguousarray(np.tile(w, (8, 1)), dtype=np.int16)


def gather64(nc, out_ap, in_ap, idxs_ap, num_idxs, elem_size, elem_step, queue_num):
    """nc.gpsimd.dma_gather minus the `elem_size_bytes % 256 == 0` assert.

    Mirrors bass.BassGpSimd.dma_gather for the DRAM-source, non-transpose,
    gen_mode=0 case.  elem_step (the idx stride) must still be a multiple
    of 256B; elem_size may be any positive size (64B verified on HW).
    """
    gp = nc.gpsimd
    gp._assert_queue_num(queue_num)
    assert idxs_ap.dtype == mybir.dt.int16
    assert in_ap.dtype == out_ap.dtype
    from concourse import ap_utils
    from concourse.bass import exact_div

    assert ap_utils.ap_is_contiguous(in_ap.ap[1:])
    assert ap_utils.ap_is_contiguous(out_ap.ap[1:])
    assert ap_utils.ap_is_contiguous(idxs_ap.ap[1:])
    assert in_ap.ap[-1][1] == out_ap.ap[-1][1] == elem_size
    assert out_ap.ap[0][1] * out_ap.ap[1][1] == ((num_idxs + 127) // 128) * 128
    assert in_ap.ap[0][0] == elem_step
    stride_bytes = elem_step * mybir.dt.size(in_ap.dtype)
    stride_bytes_256 = exact_div(stride_bytes, 256)
    assert stride_bytes_256 < 256
    _in_ap = gp.lower_ap_dma(in_ap, for_custom_bir_dma=True)
    _idxs_ap = gp.lower_ap(idxs_ap)
    _out_ap = gp.lower_ap(out_ap)
    return gp.add_instruction(
        mybir.InstDMAGatherAnt(
            name=nc.get_next_instruction_name(),
            ins=[*_in_ap, _idxs_ap, gp.lower_val_access(gp.to_reg(num_idxs))],
            outs=[_out_ap],
            transpose=False,
            num_idxs=num_idxs,
            elem_size=elem_size,
            stride_bytes_256=stride_bytes_256,
            gen_mode=0,
            single_packet=True,
            queue_num=queue_num,
            sbuf_tokens_per_rank=0,
            sbuf_free_dim_per_rank=0,
            sbuf_free_dim_pad_per_rank=0,
            sbuf_byte_offset=0,
        )
    )


def _emit_dinv(nc, pool, deg_d, p, w, name):
    """deg (int32 [p, w]) -> dinv = 1/sqrt(deg) with a Newton step."""
    deg_t = pool.tile([p, w], mybir.dt.int32, name=f"{name}_i", tag="dinv_i")
    degf_t = pool.tile([p, w], mybir.dt.float32, name=f"{name}_f", tag="dinv_f")
    r_t = pool.tile([p, w], mybir.dt.float32, name=f"{name}_r", tag="dinv_r")
    s_t = pool.tile([p, w], mybir.dt.float32, name=f"{name}_s", tag="dinv_s")
    dinv_t = pool.tile([p, w], mybir.dt.float32, name=f"{name}_v", tag="dinv_v")
    nc.sync.dma_start(out=deg_t[:], in_=deg_d[:])
    nc.vector.tensor_copy(out=degf_t[:], in_=deg_t[:])
    nc.vector.reciprocal(out=r_t[:], in_=degf_t[:])
    # Newton: r <- r * (2 - d * r) computed as -(r * (d*r - 2))
    nc.vector.tensor_tensor(out=s_t[:], in0=degf_t[:], in1=r_t[:], op=mybir.AluOpType.mult)
    nc.vector.tensor_scalar_add(out=s_t[:], in0=s_t[:], scalar1=-2.0)
    nc.vector.tensor_tensor(out=s_t[:], in0=s_t[:], in1=r_t[:], op=mybir.AluOpType.mult)
    nc.vector.tensor_scalar_mul(out=s_t[:], in0=s_t[:], scalar1=-1.0)
    nc.scalar.sqrt(dinv_t[:], s_t[:])
    return dinv_t


# ---------------------------------------------------------------- layout


class Layout:
    """Slot/instruction structure for launch B (static across cores).

    Slot stream per chunk k: cells (t, k) for t = 0..97, each padded to a
    64-slot boundary; the stream is cut into 1024-idx instructions at
    arbitrary cell positions.  Per (t, k) the cell maps to a static list
    of (instr, batch-in-instr, lo, hi) 64-granular matmul pieces.
    """

    def __init__(self, cfg: Cfg, ncell):
        self.cfg = cfg
        self.ncell = ncell            # [ntiles, n_chunks] slots per cell (64-mult)
        T, K, NI = cfg.ntiles, cfg.n_chunks, cfg.ni
        self.cell_start = np.zeros((T, K), dtype=np.int64)  # global slot idx
        self.k_slots = []             # padded stream length per chunk
        self.k_base = []              # global slot base per chunk
        self.instrs = []              # (k, slot0_in_k, nidx, batch_base)
        self.tk_pieces = {}
        base = 0
        B = 0
        for k in range(K):
            s = 0
            for t in range(T):
                self.cell_start[t, k] = base + s
                s += int(ncell[t, k])
            spad = ((s + NI - 1) // NI) * NI
            self.k_base.append(base)
            self.k_slots.append(spad)
            for i in range(spad // NI):
                self.instrs.append((k, i * NI, NI, B))
                B += NI // 128
            base += spad
        self.NS = base
        self.NB = B
        self.NI_n = len(self.instrs)
        ipk = [self.k_slots[k] // NI for k in range(K)]
        i_base = np.cumsum([0] + ipk)
        for k in range(K):
            for t in range(T):
                s0 = int(self.cell_start[t, k]) - self.k_base[k]
                s1 = s0 + int(ncell[t, k])
                pieces = []
                s = s0
                while s < s1:
                    b = s // 128
                    lo = s - b * 128
                    hi = min(s1 - b * 128, 128)
                    i = int(i_base[k]) + b * 128 // NI
                    bi = b - (b * 128 // NI) * (NI // 128)
                    assert lo in (0, 64) and hi in (64, 128)
                    pieces.append((i, bi, lo, hi))
                    s = b * 128 + hi
                self.tk_pieces[(t, k)] = pieces


def make_layout(cfg: Cfg, counts):
    """counts: [n_cores, ntiles, n_chunks] edge counts -> static Layout."""
    mx = counts.max(axis=0)
    ncell = np.maximum((mx + 63) // 64, 1) * 64
    return Layout(cfg, ncell)


# ---------------------------------------------------------------- launch A


def build_launch_a(cfg: Cfg):
    nc = bacc.Bacc(
        "TRN2", target_bir_lowering=False, debug=False, num_devices=cfg.n_cores
    )
    T, K, F = cfg.ntiles, cfg.in_dim, cfg.net_dim
    xT_d = nc.dram_tensor("xT", [T, K, 128], mybir.dt.bfloat16, kind="ExternalInput")
    w_d = nc.dram_tensor("w", [K, F], mybir.dt.bfloat16, kind="ExternalInput")
    deg_d = nc.dram_tensor("deg", [128, T], mybir.dt.int32, kind="ExternalInput")
    g_d = nc.dram_tensor("g", [cfg.npad, F], mybir.dt.bfloat16, kind="ExternalOutput")
    dv_d = nc.dram_tensor("dinv1", [cfg.npad], mybir.dt.float32, kind="ExternalOutput")

    with tile.TileContext(nc) as tc:
        with (
            tc.tile_pool(name="const", bufs=1) as cpool,
            tc.tile_pool(name="xin", bufs=4) as xpool,
            tc.tile_pool(name="gout", bufs=4) as gpool,
            tc.tile_pool(name="ph", bufs=4, space="PSUM") as php,
        ):
            w_t = cpool.tile([K, F], mybir.dt.bfloat16)
            nc.sync.dma_start(out=w_t[:], in_=w_d[:])
            dinv_t = _emit_dinv(nc, cpool, deg_d, 128, T, "dinv")

            GA = 8
            for q in range((T + GA - 1) // GA):
                t0 = q * GA
                tn = min(GA, T - t0)
                xT_t = xpool.tile([128, GA, 128], mybir.dt.bfloat16, tag="x")
                nc.sync.dma_start(
                    out=xT_t[:, :tn, :],
                    in_=xT_d[t0 : t0 + tn].rearrange("a f c -> f a c"),
                )
                g_t = gpool.tile([128, GA, F], mybir.dt.bfloat16, tag="g")
                for j in range(tn):
                    h_p = php.tile([128, F], mybir.dt.float32, tag="h")
                    nc.tensor.matmul(
                        h_p[:], xT_t[:, j, :], w_t[:], start=True, stop=True
                    )
                    # per-partition dinv scale on DVE (ACT copies are ~7x
                    # slower and were launch A's wall)
                    nc.vector.tensor_tensor(
                        out=g_t[:, j, :],
                        in0=h_p[:],
                        in1=dinv_t[:, t0 + j : t0 + j + 1].broadcast_to((128, F)),
                        op=mybir.AluOpType.mult,
                    )
                nc.sync.dma_start(
                    out=g_d[t0 * 128 : (t0 + tn) * 128, :].rearrange(
                        "(a p) c -> p a c", p=128
                    ),
                    in_=g_t[:, :tn, :],
                )

            nc.sync.dma_start(
                out=dv_d.ap().rearrange("(t p) -> p t", p=128), in_=dinv_t[:]
            )
    nc.compile()
    return nc


# ---------------------------------------------------------------- launch B


def build_launch_b(cfg: Cfg, lay: Layout):
    nc = bacc.Bacc(
        "TRN2",
        target_bir_lowering=False,
        debug=False,
        num_devices=cfg.n_cores,
        num_swdge_queues=4,
    )
    F, GEL, NI = cfg.net_dim, cfg.gel, cfg.ni
    NB, NIN = lay.NB, lay.NI_n
    NBPI = NI // 128  # batches per instr

    g_d = nc.dram_tensor("g", [cfg.n, cfg.row], mybir.dt.bfloat16, kind="ExternalInput")
    src_d = nc.dram_tensor(
        "src_i", [NIN, 128, NI // 16], mybir.dt.int16, kind="ExternalInput"
    )
    dstv_d = nc.dram_tensor("dstv", [128, NB], mybir.dt.bfloat16, kind="ExternalInput")
    iota_d = nc.dram_tensor("iota", [128, 128], mybir.dt.bfloat16, kind="ExternalInput")
    dv1_d = nc.dram_tensor("dinv1", [cfg.npad], mybir.dt.float32, kind="ExternalInput")
    bg_d = nc.dram_tensor("bg", [F, 1], mybir.dt.float32, kind="ExternalInput")
    wd_d = nc.dram_tensor("wd", [F, F], mybir.dt.float32, kind="ExternalInput")
    bd_d = nc.dram_tensor("bd", [F, 1], mybir.dt.float32, kind="ExternalInput")
    out_d = nc.dram_tensor("out", [F, cfg.npad], mybir.dt.float32, kind="ExternalOutput")

    # instr id -> list of (t, pieces) fully contained; a cell's pieces may
    # span two instrs only via its batch fragments, which are per-instr
    # anyway, so group pieces by their instr.
    by_instr = [[] for _ in range(NIN)]
    for k in range(cfg.n_chunks):
        for t in range(cfg.ntiles):
            for (i, bi, lo, hi) in lay.tk_pieces[(t, k)]:
                by_instr[i].append((t, k, bi, lo, hi))

    with tile.TileContext(nc) as tc:
        with (
            tc.tile_pool(name="const", bufs=1) as cpool,
            tc.tile_pool(name="idx", bufs=24) as ipool,
            tc.tile_pool(name="msg", bufs=40) as mpool,
            tc.tile_pool(name="oh", bufs=16) as opool,
            tc.tile_pool(name="epi", bufs=2) as epool,
            tc.tile_pool(name="acc", bufs=6, space="PSUM") as apool,
            tc.tile_pool(name="h2", bufs=2, space="PSUM") as hpool,
        ):
            iota_t = cpool.tile([128, 128], mybir.dt.bfloat16)
            nc.sync.dma_start(out=iota_t[:], in_=iota_d[:])
            dstv_t = cpool.tile([128, NB], mybir.dt.bfloat16)
            nc.sync.dma_start(out=dstv_t[:], in_=dstv_d[:])
            bg_t = cpool.tile([F, 1], mybir.dt.float32)
            nc.sync.dma_start(out=bg_t[:], in_=bg_d[:])
            wd_t = cpool.tile([F, F], mybir.dt.float32)
            nc.sync.dma_start(out=wd_t[:], in_=wd_d[:])
            bd_t = cpool.tile([F, 1], mybir.dt.float32)
            nc.sync.dma_start(out=bd_t[:], in_=bd_d[:])
            acc_t = cpool.tile([F, cfg.npad], mybir.dt.float32, name="accsb")
            nc.vector.memset(acc_t[:], 0.0)
            # dinv feature-major [32, npad], loaded after the first gathers
            # are queued so the preamble doesn't delay them.
            dinv_t = cpool.tile([F, cfg.npad], mybir.dt.float32, name="dinvfm")

            # per-(t,k) accumulation state: psum tile + piece countdown
            npieces = {
                (t, k): len(lay.tk_pieces[(t, k)])
                for k in range(cfg.n_chunks)
                for t in range(cfg.ntiles)
            }
            done = {key: 0 for key in npieces}
            psum_of = {}

            # epilogue blocks (4 tiles each), emitted as soon as all their
            # tiles' cells have folded into acc so the tail doesn't serialize
            # after the last gather
            BL = 512
            NQ = (cfg.npad + BL - 1) // BL
            tile_cells = [0] * cfg.ntiles
            blk_left = [min(4 * (q + 1), cfg.ntiles) - 4 * q for q in range(NQ)]
            emitted = [False] * NQ

            def epilogue_block(q):
                c0 = q * BL
                W = min(BL, cfg.npad - c0)
                h1_t = epool.tile([F, BL], mybir.dt.float32, tag="h1", name="h1")
                nc.vector.tensor_tensor(
                    out=h1_t[:, :W],
                    in0=acc_t[:, c0 : c0 + W],
                    in1=dinv_t[:, c0 : c0 + W],
                    op=mybir.AluOpType.mult,
                )
                r1_t = epool.tile([F, BL], mybir.dt.float32, tag="r1", name="r1")
                nc.scalar.activation(
                    r1_t[:, :W], h1_t[:, :W],
                    mybir.ActivationFunctionType.Relu, bias=bg_t[:],
                )
                h2_p = hpool.tile([F, BL], mybir.dt.float32, tag="h2", name="h2")
                nc.tensor.matmul(
                    h2_p[:, :W], wd_t[:], r1_t[:, :W], start=True, stop=True
                )
                o_t = epool.tile([F, BL], mybir.dt.float32, tag="o", name="o")
                nc.scalar.activation(
                    o_t[:, :W], h2_p[:, :W],
                    mybir.ActivationFunctionType.Relu, bias=bd_t[:],
                )
                nc.sync.dma_start(out=out_d[:, c0 : c0 + W], in_=o_t[:, :W])
                emitted[q] = True

            for i, (k, s0, nidx, bb) in enumerate(lay.instrs):
                idx_t = ipool.tile([128, NI // 16], mybir.dt.int16, tag="idx")
                nc.sync.dma_start(out=idx_t[:], in_=src_d[i])
                msg_t = mpool.tile([128, NBPI, GEL], mybir.dt.bfloat16, tag="m")
                gather64(
                    nc,
                    msg_t[:],
                    g_d[k * cfg.chunk : (k + 1) * cfg.chunk, 0:GEL],
                    idx_t[:],
                    nidx,
                    GEL,
                    cfg.row,
                    queue_num=i % 4,
                )
                oh_t = opool.tile([128, NBPI, 128], mybir.dt.bfloat16, tag="oh")
                nc.vector.tensor_tensor(
                    out=oh_t[:],
                    in0=iota_t[:].unsqueeze(1).broadcast_to((128, NBPI, 128)),
                    in1=dstv_t[:, bb : bb + NBPI]
                    .unsqueeze(2)
                    .broadcast_to((128, NBPI, 128)),
                    op=mybir.AluOpType.is_equal,
                )

                if i == 1:
                    for f in range(F):
                        nc.sync.dma_start(
                            out=dinv_t[f : f + 1, :], in_=dv1_d.ap().unsqueeze(0)
                        )

                for (t, k2, bi, lo, hi) in by_instr[i]:
                    key = (t, k2)
                    if key not in psum_of:
                        # full PSUM bank per open accumulator: a start=True
                        # while another group is open in the same bank
                        # destroys it on HW.
                        psum_of[key] = apool.tile(
                            [128, 512], mybir.dt.float32, tag="acc",
                            name=f"acc_{t}_{k2}",
                        )
                    p_t = psum_of[key]
                    first = done[key] == 0
                    done[key] += 1
                    last = done[key] == npieces[key]
                    nc.tensor.matmul(
                        p_t[0:F, 0:128],
                        msg_t[lo:hi, bi, :],
                        oh_t[lo:hi, bi, :],
                        start=first,
                        stop=last,
                    )
                    if last:
                        nc.vector.tensor_tensor(
                            out=acc_t[:, t * 128 : (t + 1) * 128],
                            in0=acc_t[:, t * 128 : (t + 1) * 128],
                            in1=p_t[0:F, 0:128],
                            op=mybir.AluOpType.add,
                        )
                        del psum_of[key]
                        tile_cells[t] += 1
                        if tile_cells[t] == cfg.n_chunks:
                            q = t // 4
                            blk_left[q] -= 1
                            if blk_left[q] == 0:
                                epilogue_block(q)

            for q in range(NQ):
                if not emitted[q]:
                    epilogue_block(q)
    nc.compile()
    return nc


# ---------------------------------------------------------------- host side


def host_prep(x, edge_index, W_gcn, b_gcn, W_dense, b_dense, cfg: Cfg):
    n, nloc = cfg.n, cfg.nloc
    row = np.asarray(edge_index[0]).astype(np.int64)
    col = np.asarray(edge_index[1]).astype(np.int64)
    deg = (np.bincount(col, minlength=n) + 1).astype(np.int32)  # + self-loop

    W_gcn = _f32(W_gcn)
    b_gcn = _f32(b_gcn).reshape(cfg.net_dim, 1)
    W_dense = _f32(W_dense)
    b_dense = _f32(b_dense).reshape(cfg.net_dim, 1)
    x = _f32(x)

    import ml_dtypes

    iota = np.tile(np.arange(128), (128, 1)).astype(ml_dtypes.bfloat16)

    # ---- per-core edge sets (dst-sharded) + self loops.  Node -> tile
    # assignment is rebalanced per core (greedy 4-D bin packing on per-chunk
    # degrees) so the (tile, chunk) cells are near-uniform: the layout pads
    # each cell to the 64-aligned max across cores, so balance = fewer pad
    # slots = fewer gather descriptors (the kernel's wall).
    owner = col // nloc
    per_core = []
    perms = []
    counts = np.zeros((cfg.n_cores, cfg.ntiles, cfg.n_chunks), dtype=np.int64)
    for c in range(cfg.n_cores):
        m = owner == c
        srcs = row[m]
        dstl = col[m] - c * nloc
        loop = np.arange(nloc, dtype=np.int64)
        srcs = np.concatenate([srcs, loop + c * nloc])
        dstl = np.concatenate([dstl, loop])
        k = srcs // cfg.chunk

        d4 = np.zeros((nloc, cfg.n_chunks), dtype=np.int64)
        np.add.at(d4, (dstl, k), 1)
        # LPT on total degree (balances tile totals), then chunk-mix swaps
        order_n = np.argsort(-d4.sum(axis=1), kind="stable")
        cell = np.zeros((cfg.ntiles, cfg.n_chunks), dtype=np.int64)
        fill = np.zeros(cfg.ntiles, dtype=np.int64)
        tile_of = np.empty(nloc, dtype=np.int64)
        BIG = 1 << 30
        for i in order_n:
            cand = cell.sum(axis=1) + np.where(fill >= 128, BIG, 0)
            t_best = int(np.argmin(cand))
            tile_of[i] = t_best
            fill[t_best] += 1
            cell[t_best] += d4[i]

        # swap-descent: push the worst (tile, chunk) cells toward the mean
        # (each pad-to-64 slot above the cross-core max is a wasted gather
        # descriptor on the kernel's critical path)
        nodes_by_tile = [list(np.nonzero(tile_of == t)[0]) for t in range(cfg.ntiles)]
        rng = np.random.default_rng(c)
        # push every cell under the next 64-slot boundary
        target = (int(np.ceil(cell.sum() / cell.size)) // 64 + 1) * 64 - 8
        stuck = set()
        for _ in range(40000):
            flat = cell.copy()
            for (ts, ks) in stuck:
                flat[ts, ks] = -1
            t1, k1 = np.unravel_index(int(np.argmax(flat)), cell.shape)
            if cell[t1, k1] <= target:
                break
            na = nodes_by_tile[t1]
            sa = np.argsort(-(2 * d4[na, k1] - d4[na].sum(axis=1)))
            a_cands = np.asarray(na)[sa[:12]]
            improved = False
            for t2 in np.argsort(cell[:, k1])[:24]:
                if improved:
                    break
                nb = nodes_by_tile[t2]
                sb = np.argsort(2 * d4[nb, k1] - d4[nb].sum(axis=1))
                b_cands = np.asarray(nb)[sb[:12]]
                for a in a_cands:
                    if improved:
                        break
                    for b in b_cands:
                        c1n = cell[t1] - d4[a] + d4[b]
                        c2n = cell[t2] - d4[b] + d4[a]
                        if max(c1n.max(), c2n.max()) < cell[t1, k1]:
                            cell[t1] = c1n
                            cell[t2] = c2n
                            nodes_by_tile[t1].remove(a)
                            nodes_by_tile[t2].remove(b)
                            nodes_by_tile[t1].append(b)
                            nodes_by_tile[t2].append(a)
                            tile_of[a] = t2
                            tile_of[b] = t1
                            improved = True
                            break
            if not improved:
                stuck.add((t1, k1))
                if len(stuck) > 400:
                    break

        newpos = np.empty(nloc, dtype=np.int64)
        fill[:] = 0
        for i in range(nloc):
            t = tile_of[i]
            newpos[i] = t * 128 + fill[t]
            fill[t] += 1
        perms.append(newpos)

        dstl = newpos[dstl]
        t = dstl >> 7
        np.add.at(counts[c], (t, k), 1)
        per_core.append((srcs, dstl, t, k))

    lay = make_layout(cfg, counts)
    lay.perms = perms

    in_a, in_b = [], []
    for c in range(cfg.n_cores):
        srcs, dstl, t, k = per_core[c]
        newpos = perms[c]
        dpad = np.ones(cfg.npad, dtype=np.int32)
        dpad[newpos] = deg[c * nloc : (c + 1) * nloc]
        deg_a = np.ascontiguousarray(dpad.reshape(cfg.ntiles, 128).T)  # [128, T]

        xpad = np.zeros((cfg.npad, cfg.in_dim), dtype=np.float32)
        xpad[newpos] = x[c * nloc : (c + 1) * nloc]
        xT3 = np.ascontiguousarray(
            xpad.reshape(cfg.ntiles, 128, cfg.in_dim).transpose(0, 2, 1)
        ).astype(ml_dtypes.bfloat16)
        in_a.append({"xT": xT3, "w": W_gcn.astype(ml_dtypes.bfloat16), "deg": deg_a})

        # ---- slot assignment into the v3 stream
        src_slots = np.zeros(lay.NS, dtype=np.int64)
        dst_slots = np.full(lay.NS, -1.0, dtype=np.float64)
        order = np.lexsort((t, k))
        ts, ks = t[order], k[order]
        so = (srcs[order] - ks * cfg.chunk).astype(np.int64)
        do = (dstl[order] & 127).astype(np.int64)
        grp = ks * cfg.ntiles + ts
        uq, starts_, cnts_ = np.unique(grp, return_index=True, return_counts=True)
        rank = np.arange(grp.size) - np.repeat(starts_, cnts_)
        slot = lay.cell_start[ts, ks] + rank
        src_slots[slot] = so
        dst_slots[slot] = do

        src_i = np.zeros((lay.NI_n, 128, cfg.ni // 16), dtype=np.int16)
        dstv = np.empty((128, lay.NB), dtype=np.float64)
        for i, (kk, s0, nidx, bb) in enumerate(lay.instrs):
            g0 = lay.k_base[kk] + s0
            seg = src_slots[g0 : g0 + nidx]
            src_i[i] = wrap16(seg)
            dseg = dst_slots[g0 : g0 + nidx]
            nbt = nidx // 128
            dstv[:, bb : bb + nbt] = dseg.reshape(nbt, 128).T
        dstv = np.ascontiguousarray(dstv).astype(ml_dtypes.bfloat16)

        in_b.append(
            {
                "src_i": src_i,
                "dstv": dstv,
                "iota": iota,
                "bg": b_gcn,
                "wd": W_dense,
                "bd": b_dense,
            }
        )
    return in_a, in_b, lay


def assemble_table(res_a, cfg: Cfg, lay: Layout):
    import ml_dtypes

    tab = np.zeros((cfg.n, cfg.row), dtype=ml_dtypes.bfloat16)
    for c in range(cfg.n_cores):
        tab[c * cfg.nloc : (c + 1) * cfg.nloc, 0 : cfg.net_dim] = res_a[c]["g"][
            lay.perms[c]
        ]
    return tab


def assemble_out(res_b, cfg: Cfg, lay: Layout):
    return np.ascontiguousarray(
        np.concatenate(
            [res_b[c]["out"].T[lay.perms[c]] for c in range(cfg.n_cores)], axis=0
        )
    ).astype(np.float32)


def _add_table(in_b, table, res_a, cfg: Cfg):
    for c, m in enumerate(in_b):
        m["g"] = table
        m["dinv1"] = np.ascontiguousarray(res_a[c]["dinv1"], dtype=np.float32)


def kernel(x, edge_index, W_gcn, b_gcn, W_dense, b_dense):
    cfg = FULL
    in_a, in_b, lay = host_prep(x, edge_index, W_gcn, b_gcn, W_dense, b_dense, cfg)
    nc_a = build_launch_a(cfg)
    nc_b = build_launch_b(cfg, lay)
    core_ids = list(range(cfg.n_cores))
    res_a = run_bass_kernel_spmd(nc_a, in_a, core_ids).results
    table = assemble_table(res_a, cfg, lay)
    _add_table(in_b, table, res_a, cfg)
    res_b = run_bass_kernel_spmd(nc_b, in_b, core_ids).results
    return assemble_out(res_b, cfg, lay)
